# revision 2
# baseline (speedup 1.0000x reference)
"""Trainium2 Bass kernel for nn_ContrastiveLoss (NT-Xent-style loss with
tag/document masking).

Strategy (8 NeuronCores, SPMD):
  - Rows of the 8192x8192 similarity matrix are sharded: core c owns 1024 rows.
  - The host L2-normalizes the embeddings, casts to bf16 and ships them
    TRANSPOSED ([256, 8192]) with columns ROLLED so the core's own 1024 rows
    sit at columns [0:1024] (pure SPMD: lhsT is always columns [0:1024] and
    the positive-pair partner of local row m is always column 4096+m).
  - Tag-equality masking is FUSED INTO THE MATMUL: a third K-tile contracts
    -30*onehot(tag(row)) against onehot(tag(col)), so PSUM holds
    sim - 30*[tag_eq] and exp(2*sim - 60*[tag_eq]) underflows to 0 in fp16,
    killing tag-equal pairs.
  - The 8192 columns of each 128-row tile are processed in 4 "quads" of 2048
    columns (4 PSUM banks).  Two [128,2048] PSUM tiles double-buffer the
    quads so TensorE never waits for ScalarE.
  - ONE wide exp per quad (ACT, free dim 2048) and ONE wide DVE
    scalar_tensor_tensor per quad: (docs != doc_row) * E with accum_out
    produces the masked denominator partial directly (tag-equal entries are
    already 0, so the doc-inequality mask alone realizes the AND-mask).
  - Per row tile the device also extracts the raw partner diagonal from PSUM.
    The host does the final ln()/assembly of the scalar loss.
"""

import sys

for _p in ("/opt/trn_rl_repo", "/root/.axon_site/_ro/trn_rl_repo"):
    if _p not in sys.path:
        sys.path.insert(0, _p)

from contextlib import ExitStack

import ml_dtypes
import numpy as np

from concourse import bacc, mybir, tile
from concourse.bass_utils import run_bass_kernel_spmd

F32 = mybir.dt.float32
F16 = mybir.dt.float16
BF16 = mybir.dt.bfloat16
BF16NP = ml_dtypes.bfloat16

P = 128          # SBUF partitions
B = 4096         # batch
D = 256          # embedding dim
N = 2 * B        # 8192 rows/cols of the similarity matrix
CORES = 8
ROWS_PER_CORE = N // CORES      # 1024
NI = ROWS_PER_CORE // P         # 8 row tiles per core
CH = 512                        # one PSUM bank of fp32
QW = 4 * CH                     # quad width: 4 banks processed per exp
NQ = N // QW                    # 4 quads per row tile
KT = D // P                     # 2 contraction tiles for sim
TEMP_SCALE = 2.0                # 1 / TEMPERATURE
MASK_PEN = 30.0                 # tag-mask penalty fused into the matmul


def _build_program():
    nc = bacc.Bacc(None, target_bir_lowering=False)

    reps_d = [nc.declare_dram_parameter(f"reps{k}", [P, N], BF16,
                                        isOutput=False) for k in range(KT)]
    docsb_d = nc.declare_dram_parameter("docsb", [P, N], F16, isOutput=False)
    docst_d = nc.declare_dram_parameter("docst", [P, NI], F32, isOutput=False)
    tagoh_d = nc.declare_dram_parameter("tagoh", [P, ROWS_PER_CORE], BF16,
                                        isOutput=False)
    tagohc_d = nc.declare_dram_parameter("tagohc", [P, N], BF16, isOutput=False)
    ident_d = nc.declare_dram_parameter("ident", [P, P], F16, isOutput=False)
    out_d = nc.declare_dram_parameter("out", [P, 5 * NI], F32, isOutput=True)

    Exp = mybir.ActivationFunctionType.Exp
    mult = mybir.AluOpType.mult
    not_equal = mybir.AluOpType.not_equal

    with tile.TileContext(nc) as tc, ExitStack() as ctx:
        persist = ctx.enter_context(tc.tile_pool(name="persist", bufs=1))
        repsT = [persist.tile([P, N], BF16, tag=f"repsT{k}", name=f"repsT{k}")
                 for k in range(KT)]
        docs_b = persist.tile([P, N], F16, tag="docs_b")
        tagohc = persist.tile([P, N], BF16, tag="tagohc")
        tagoh = persist.tile([P, ROWS_PER_CORE], BF16, tag="tagoh")
        docst = persist.tile([P, NI], F32, tag="docst")
        ident = persist.tile([P, P], F16, tag="ident")
        v_sb = persist.tile([P, 5 * NI], F32, tag="v_sb")

        nc.sync.dma_start(docst[:], docst_d[:])
        nc.sync.dma_start(ident[:], ident_d[:])
        nc.sync.dma_start(tagoh[:], tagoh_d[:])
        # split the big column loads so the first quads' inputs land first
        for cc in range(NQ):
            qs = slice(cc * QW, (cc + 1) * QW)
            for k in range(KT):
                nc.sync.dma_start(repsT[k][:, qs], reps_d[k][:, qs])
            nc.sync.dma_start(tagohc[:, qs], tagohc_d[:, qs])
            nc.sync.dma_start(docs_b[:, qs], docsb_d[:, qs])

        lhs = [repsT[0], repsT[1], tagoh]
        rhs = [repsT[0], repsT[1], tagohc]
        NK = 3
        with (
            tc.tile_pool(name="work", bufs=3) as work,
            tc.tile_pool(name="junkp", bufs=2) as junkp,
            tc.tile_pool(name="psm", bufs=2, space="PSUM") as psm,
        ):
            for i in range(NI):
                ms = slice(i * P, (i + 1) * P)
                for q in range(NQ):
                    S2 = psm.tile([P, QW], F32, tag="S2", name=f"S2_{i}_{q}")
                    for k in range(NK):
                        for c in range(4):
                            cps = slice(c * CH, (c + 1) * CH)
                            cols = slice(q * QW + c * CH, q * QW + (c + 1) * CH)
                            nc.tensor.matmul(
                                S2[:, cps],
                                lhs[k][:, ms],
                                rhs[k][:, cols],
                                start=(k == 0),
                                stop=(k == NK - 1),
                            )
                    Et = work.tile([P, QW], F16, tag="Et")
                    nc.scalar.activation(Et[:], S2[:], Exp, scale=TEMP_SCALE)
                    if q == 2:
                        # partner diagonal: global col 4096+i*128+p ->
                        # offset i*128 inside quad 2
                        junkd = junkp.tile([P, P], F16, tag="junkd")
                        nc.vector.scalar_tensor_tensor(
                            junkd[:], ident[:], 1.0,
                            S2[:, i * P:(i + 1) * P],
                            mult, mult, accum_out=v_sb[:, 5 * i + 4:5 * i + 5],
                        )
                    junk = junkp.tile([P, QW], F16, tag="junk")
                    nc.vector.scalar_tensor_tensor(
                        junk[:], docs_b[:, q * QW:(q + 1) * QW],
                        docst[:, i:i + 1], Et[:],
                        not_equal, mult,
                        accum_out=v_sb[:, 5 * i + q:5 * i + q + 1],
                    )

            nc.sync.dma_start(out_d[:], v_sb[:])

    nc.compile()
    return nc


_NC_CACHE = []


def _get_nc():
    if not _NC_CACHE:
        _NC_CACHE.append(_build_program())
    return _NC_CACHE[0]


def _prepare_inputs(emb_i, emb_j, tags, document_ids):
    emb = np.concatenate(
        [np.asarray(emb_i), np.asarray(emb_j)], axis=0).astype(np.float32)
    emb /= np.linalg.norm(emb, axis=1, keepdims=True)
    repsT = np.ascontiguousarray(emb.T)                            # [256, 8192]
    tags2 = np.concatenate([tags, tags]).astype(np.int64)          # [8192]
    docs2 = np.concatenate([document_ids, document_ids]).astype(np.float16)
    ident = np.eye(P, dtype=np.float16)

    # onehot(tag) with class dim padded to 128 partitions (tags < 100)
    ohc_full = np.zeros((P, N), dtype=BF16NP)
    ohc_full[tags2, np.arange(N)] = BF16NP(1.0)

    in_maps = []
    for c in range(CORES):
        r = c * ROWS_PER_CORE
        roll = np.r_[r:N, 0:r]
        dv = docs2[roll]
        ohc = np.ascontiguousarray(ohc_full[:, roll])
        rT = np.ascontiguousarray(repsT[:, roll]).astype(BF16NP)
        m = {
            "docsb": np.ascontiguousarray(
                np.broadcast_to(dv.reshape(1, N), (P, N))),
            "docst": np.ascontiguousarray(
                dv[:ROWS_PER_CORE].reshape(NI, P).T.astype(np.float32)),
            "tagoh": np.ascontiguousarray(
                (ohc[:, :ROWS_PER_CORE].astype(np.float32)
                 * -MASK_PEN).astype(BF16NP)),
            "tagohc": ohc,
            "ident": ident,
        }
        for k in range(KT):
            m[f"reps{k}"] = np.ascontiguousarray(rT[k * P:(k + 1) * P, :])
        in_maps.append(m)
    return in_maps


def _assemble_loss(results):
    total = 0.0
    for c in range(CORES):
        o = np.asarray(results[c]["out"]).astype(np.float64)
        o = o.reshape(P, NI, 5)
        denom = o[:, :, 0:4].sum(axis=2) + 0.1
        sdiag = o[:, :, 4] + MASK_PEN        # undo fused tag penalty
        v = np.log(denom) - TEMP_SCALE * sdiag
        total += v.sum()
    return np.float32(total / N)


def kernel(emb_i, emb_j, tags, num_classes, document_ids):
    nc = _get_nc()
    in_maps = _prepare_inputs(emb_i, emb_j, tags, document_ids)
    res = run_bass_kernel_spmd(nc, in_maps, list(range(CORES)))
    return _assemble_loss(res.results)


# revision 6
# speedup vs baseline: 1.1371x; 1.1371x over previous
"""Trainium2 Bass kernel for nn_ContrastiveLoss (NT-Xent-style loss with
tag/document masking).

Strategy (8 NeuronCores, SPMD):
  - Rows of the 8192x8192 similarity matrix are sharded: core c owns 1024 rows.
  - The host L2-normalizes the embeddings, casts to bf16 and ships them
    TRANSPOSED ([256, 8192]) with columns ROLLED so the core's own 1024 rows
    sit at columns [0:1024] (pure SPMD: lhsT is always columns [0:1024] and
    the positive-pair partner of local row m is always column 4096+m).
  - Tag-equality masking is FUSED INTO THE MATMUL: a third K-tile contracts
    -30*onehot(tag(row)) against onehot(tag(col)), so PSUM holds
    sim - 30*[tag_eq] and exp(2*sim - 60*[tag_eq]) underflows to 0 in fp16,
    killing tag-equal pairs.
  - The 8192 columns of each 128-row tile are processed in 4 "quads" of 2048
    columns (4 PSUM banks).  Two [128,2048] PSUM tiles double-buffer the
    quads so TensorE never waits for ScalarE.
  - ONE wide exp per quad (ACT, free dim 2048) and ONE wide DVE
    scalar_tensor_tensor per quad: (docs != doc_row) * E with accum_out
    produces the masked denominator partial directly (tag-equal entries are
    already 0, so the doc-inequality mask alone realizes the AND-mask).
  - Per row tile the device also extracts the raw partner diagonal from PSUM.
    The host does the final ln()/assembly of the scalar loss.
"""

import sys

for _p in ("/opt/trn_rl_repo", "/root/.axon_site/_ro/trn_rl_repo"):
    if _p not in sys.path:
        sys.path.insert(0, _p)

from contextlib import ExitStack

import ml_dtypes
import numpy as np

from concourse import bacc, mybir, tile
from concourse.bass_utils import run_bass_kernel_spmd

F32 = mybir.dt.float32
F16 = mybir.dt.float16
BF16 = mybir.dt.bfloat16
BF16NP = ml_dtypes.bfloat16

P = 128          # SBUF partitions
B = 4096         # batch
D = 256          # embedding dim
N = 2 * B        # 8192 rows/cols of the similarity matrix
CORES = 8
ROWS_PER_CORE = N // CORES      # 1024
NI = ROWS_PER_CORE // P         # 8 row tiles per core
CH = 512                        # one PSUM bank of fp32
QW = 4 * CH                     # quad width: 4 banks processed per exp
NQ = N // QW                    # 4 quads per row tile
KT = D // P                     # 2 contraction tiles for sim
TEMP_SCALE = 2.0                # 1 / TEMPERATURE
MASK_PEN = 30.0                 # tag-mask penalty fused into the matmul


def _build_program():
    nc = bacc.Bacc(None, target_bir_lowering=False)

    reps_d = [nc.declare_dram_parameter(f"reps{k}", [P, N], BF16,
                                        isOutput=False) for k in range(KT)]
    docsb_d = nc.declare_dram_parameter("docsb", [P, N], F16, isOutput=False)
    docst_d = nc.declare_dram_parameter("docst", [P, NI], F32, isOutput=False)
    tagoh_d = nc.declare_dram_parameter("tagoh", [P, ROWS_PER_CORE], BF16,
                                        isOutput=False)
    tagohc_d = nc.declare_dram_parameter("tagohc", [P, N], BF16, isOutput=False)
    ident_d = nc.declare_dram_parameter("ident", [P, P], F16, isOutput=False)
    out_d = nc.declare_dram_parameter("out", [P, 5 * NI], F32, isOutput=True)

    Exp = mybir.ActivationFunctionType.Exp
    mult = mybir.AluOpType.mult
    add = mybir.AluOpType.add
    not_equal = mybir.AluOpType.not_equal

    with tile.TileContext(nc) as tc, ExitStack() as ctx:
        persist = ctx.enter_context(tc.tile_pool(name="persist", bufs=1))
        repsT = [persist.tile([P, N], BF16, tag=f"repsT{k}", name=f"repsT{k}")
                 for k in range(KT)]
        docs_b = persist.tile([P, N], F16, tag="docs_b")
        tagohc = persist.tile([P, N], BF16, tag="tagohc")
        tagoh = persist.tile([P, ROWS_PER_CORE], BF16, tag="tagoh")
        docst = persist.tile([P, NI], F32, tag="docst")
        ident = persist.tile([P, P], F16, tag="ident")
        v_sb = persist.tile([P, 5 * NI], F32, tag="v_sb")

        # DMA order is the startup critical path: the very first matmul only
        # needs reps0[:, 0:1024] (lhsT cols 0:128 + rhs cols 0:512), so ship
        # quad 0's inputs in 1024-col slivers first, then stream the rest.
        nc.sync.dma_start(docst[:], docst_d[:])
        nc.sync.dma_start(ident[:], ident_d[:])
        H = QW // 2
        for h in range(2):
            hs = slice(h * H, (h + 1) * H)
            for k in range(KT):
                nc.sync.dma_start(repsT[k][:, hs], reps_d[k][:, hs])
            if h == 0:
                nc.sync.dma_start(tagoh[:], tagoh_d[:])
            nc.sync.dma_start(tagohc[:, hs], tagohc_d[:, hs])
        nc.sync.dma_start(docs_b[:, 0:QW], docsb_d[:, 0:QW])
        for cc in range(1, NQ):
            qs = slice(cc * QW, (cc + 1) * QW)
            for k in range(KT):
                nc.sync.dma_start(repsT[k][:, qs], reps_d[k][:, qs])
            nc.sync.dma_start(tagohc[:, qs], tagohc_d[:, qs])
            nc.sync.dma_start(docs_b[:, qs], docsb_d[:, qs])

        lhs = [repsT[0], repsT[1], tagoh]
        rhs = [repsT[0], repsT[1], tagohc]
        NK = 3
        with (
            tc.tile_pool(name="work", bufs=3) as work,
            tc.tile_pool(name="junkp", bufs=2) as junkp,
            tc.tile_pool(name="psm", bufs=2, space="PSUM") as psm,
        ):
            for i in range(NI):
                ms = slice(i * P, (i + 1) * P)
                for q in range(NQ):
                    S2 = psm.tile([P, QW], F32, tag="S2", name=f"S2_{i}_{q}")
                    # c outer / k inner: bank c only depends on input columns
                    # up to (c+1)*512, so the first matmuls start as soon as
                    # the first DMA slivers land.
                    for c in range(4):
                        cps = slice(c * CH, (c + 1) * CH)
                        cols = slice(q * QW + c * CH, q * QW + (c + 1) * CH)
                        for k in range(NK):
                            nc.tensor.matmul(
                                S2[:, cps],
                                lhs[k][:, ms],
                                rhs[k][:, cols],
                                start=(k == 0),
                                stop=(k == NK - 1),
                            )
                    Et = work.tile([P, QW], F16, tag="Et")
                    nc.scalar.activation(Et[:], S2[:], Exp, scale=TEMP_SCALE)
                    if q == 2:
                        # partner diagonal: global col 4096+i*128+p ->
                        # offset i*128 inside quad 2
                        junkd = junkp.tile([P, P], F16, tag="junkd")
                        nc.vector.scalar_tensor_tensor(
                            junkd[:], ident[:], 1.0,
                            S2[:, i * P:(i + 1) * P],
                            mult, mult, accum_out=v_sb[:, 5 * i + 4:5 * i + 5],
                        )
                    junk = junkp.tile([P, QW], F16, tag="junk")
                    nc.vector.scalar_tensor_tensor(
                        junk[:], docs_b[:, q * QW:(q + 1) * QW],
                        docst[:, i:i + 1], Et[:],
                        not_equal, mult,
                        accum_out=v_sb[:, 5 * i + q:5 * i + q + 1],
                    )

            nc.sync.dma_start(out_d[:], v_sb[:])

    nc.compile()
    return nc


_NC_CACHE = []


def _get_nc():
    if not _NC_CACHE:
        _NC_CACHE.append(_build_program())
    return _NC_CACHE[0]


def _prepare_inputs(emb_i, emb_j, tags, document_ids):
    emb = np.concatenate(
        [np.asarray(emb_i), np.asarray(emb_j)], axis=0).astype(np.float32)
    emb /= np.linalg.norm(emb, axis=1, keepdims=True)
    repsT = np.ascontiguousarray(emb.T)                            # [256, 8192]
    tags2 = np.concatenate([tags, tags]).astype(np.int64)          # [8192]
    docs2 = np.concatenate([document_ids, document_ids]).astype(np.float16)
    ident = np.eye(P, dtype=np.float16)

    # onehot(tag) with class dim padded to 128 partitions (tags < 100)
    ohc_full = np.zeros((P, N), dtype=BF16NP)
    ohc_full[tags2, np.arange(N)] = BF16NP(1.0)

    in_maps = []
    for c in range(CORES):
        r = c * ROWS_PER_CORE
        roll = np.r_[r:N, 0:r]
        dv = docs2[roll]
        ohc = np.ascontiguousarray(ohc_full[:, roll])
        rT = np.ascontiguousarray(repsT[:, roll]).astype(BF16NP)
        m = {
            "docsb": np.ascontiguousarray(
                np.broadcast_to(dv.reshape(1, N), (P, N))),
            "docst": np.ascontiguousarray(
                dv[:ROWS_PER_CORE].reshape(NI, P).T.astype(np.float32)),
            "tagoh": np.ascontiguousarray(
                (ohc[:, :ROWS_PER_CORE].astype(np.float32)
                 * -MASK_PEN).astype(BF16NP)),
            "tagohc": ohc,
            "ident": ident,
        }
        for k in range(KT):
            m[f"reps{k}"] = np.ascontiguousarray(rT[k * P:(k + 1) * P, :])
        in_maps.append(m)
    return in_maps


def _assemble_loss(results):
    total = 0.0
    for c in range(CORES):
        o = np.asarray(results[c]["out"]).astype(np.float64)
        o = o.reshape(P, NI, 5)
        denom = o[:, :, 0:4].sum(axis=2) + 0.1
        sdiag = o[:, :, 4] + MASK_PEN        # undo fused tag penalty
        v = np.log(denom) - TEMP_SCALE * sdiag
        total += v.sum()
    return np.float32(total / N)


def kernel(emb_i, emb_j, tags, num_classes, document_ids):
    nc = _get_nc()
    in_maps = _prepare_inputs(emb_i, emb_j, tags, document_ids)
    res = run_bass_kernel_spmd(nc, in_maps, list(range(CORES)))
    return _assemble_loss(res.results)
